# revision 30
# baseline (speedup 1.0000x reference)
"""Batch-all triplet loss on 8 Trainium2 NeuronCores (Bass/Tile).

Math: with d = pairwise euclidean distance matrix of the B embeddings,
  loss = sum_{i,j,k valid} relu(d[i,j] - d[i,k] + margin) / (#positive + eps)
valid <=> i != j, labels[i] == labels[j], labels[i] != labels[k]
(the other distinctness constraints are implied by the label ones).

Sharding: anchors are grouped by class; each core hosts 2 classes in two
64-row blocks (data-driven gathers keep the single SPMD program uniform).

Per core, on device:
  prep:
  - one fused matmul group produces g[r, c] = dot(x_r, x_c) - 0.5*sq_c
    - 0.5*sq_r - MASK*[class(c)==class(r)] : the X contraction runs in
    fp8 (4x128 K-chunks; the stochastic fp8 dot error ~0.02 on d is far
    inside the 2e-2 gate and halves the gating DMA bytes); a 20-row bf16
    tail chunk carries the anchor squared norm (hi+lo, stationary data
    vs moving ones), the column squared norm (hi+lo, moving data vs
    stationary ones) and 16 class-mask rows (+-240 products).
  - ACT Relu(scale=-2) then ACT Sqrt gives dmat (masked columns ~338);
    DVE tensor_scalar(-SHIFT) emits rhs_c[:, 0:B] (bf16, recentred).
  - the same structure over the partner columns gives bias[r, t] =
    d(anchor_r, t-th member of r's class) + margin - SHIFT at
    rhs_c[:, B:B+T] (bf16).
  pair loop, NT tiles of 128 (anchor,positive) pairs (software-pipelined
  so each engine's in-order stream never stalls):
  - PE replicates each pair's anchor row + bias row with a one-hot
    matmul into PSUM: rep[p, 0:B+T] = rhs_c[anchor(p), :].
  - DVE builds the bias-select mask on the fly (sel[p,t] =
    (t == pos_idx[p, tau]), 4x mode) -> scalar_tensor_tensor extracts
    bias_p; ACT Relu(scale=-1, bias=bias_p) -> o1 (bf16 SBUF);
    DVE is_gt -> o2 (bf16, 4x mode).
  - PE reduces o1 and o2 with ones-matmuls into two persistent PSUM
    accumulator regions (column-wrapped, one accumulation group each
    across all NT tiles) - the DVE reduce path is 1x-capped and slower.
  - invalid negatives contribute 0 (the +306 mask), padded pairs are
    all-zero rows with bias 0 and contribute 0 to both sums.
  tail: one strided tensor_reduce of the [1, 2x512] accumulator ->
  [1, 2] (sum, count) DMA'd out; host adds the 8 cores up.
"""

import numpy as np

import bass_rust
import concourse.bass as bass
import concourse.tile as tile
from concourse import mybir
from concourse.bass_utils import run_bass_kernel_spmd

N_CORES = 8
D_MODEL = 512
B_TOTAL = 640
N_CLASSES = 16
MARGIN = 0.3
EPS = 1e-8
RB = 64  # rows per block (max class size the device path supports)
MASKQ = 240.0  # class-mask factor; product 57600 pushes masked d to ~338
SHIFT = 32.0  # d values live in ~[27.6, 37]; recentring helps bf16
AUGR = 4 + N_CLASSES  # sqa_hi, sqa_lo, norm_hi, norm_lo, class masks
TP = 48  # padded iota/sel width (even for DVE 4x mode)
F32 = mybir.dt.float32
BF16 = mybir.dt.bfloat16
FP8 = mybir.dt.float8e4
NP_BF16 = mybir.dt.np(mybir.dt.bfloat16)
NP_FP8 = mybir.dt.np(mybir.dt.float8e4)

_PROGRAM_CACHE: dict = {}


def _split_multi_waits(nc):
    """This toolchain's walrus codegen supports only ONE sync-wait per
    instruction; Tile can emit several. Move the extra waits onto
    same-engine NoOps inserted immediately before the instruction."""
    for func in nc.m.functions:
        for block in func.blocks:
            out = []
            for inst in block.instructions:
                si = inst.sync_info
                waits = list(si.on_wait) if si else []
                if len(waits) > 1:
                    for j, w in enumerate(waits[:-1]):
                        nop = mybir.InstNoOp(
                            name=f"{inst.name}-wsplit{j}", ins=[], outs=[]
                        )
                        nop.engine = inst.engine
                        nop.sync_info = bass_rust.SyncInfo(on_wait=[w], on_update=[])
                        out.append(nop)
                    inst.sync_info = bass_rust.SyncInfo(
                        on_wait=[waits[-1]], on_update=list(si.on_update)
                    )
                out.append(inst)
            block.instructions = out


def _build_program(B: int, D: int, T: int, NT: int):
    """One SPMD program for all 8 cores; per-core behavior is data-driven."""
    nc = bass.Bass()

    # xcombo columns: [0:B) moving operand (X^T), [B:B+128) stationary
    # anchor gather, [B+128:B+128+2T) positive-partner gather, all fp8.
    # Packed DRAM layout [128, 4*W]: chunk c of the K=512 contraction
    # occupies cols [c*W:(c+1)*W) (K-row c*128+p lives in partition p).
    W = B + 128 + 2 * T
    xcombo = nc.declare_dram_parameter("xcombo", [128, 4 * W], FP8, isOutput=False)
    aug = nc.declare_dram_parameter("aug", [AUGR, W], BF16, isOutput=False)
    # one-hot stationary, shipped with only the 2*T meaningful rows
    onehot = nc.declare_dram_parameter(
        "onehot", [2 * T, NT * 128], BF16, isOutput=False
    )
    selm = nc.declare_dram_parameter("selm", [128, NT * TP], BF16, isOutput=False)
    out_d = nc.declare_dram_parameter("out", [1, 2], F32, isOutput=True)

    WR = B + T  # replicated tile: negative columns + bias columns
    L0, L1 = B, B + 128  # stationary (anchor) columns within a chunk
    P0 = B + 128  # partner columns within a chunk

    with tile.TileContext(nc) as tc:
        with (
            tc.tile_pool(name="const", bufs=1) as const,
            tc.tile_pool(name="work", bufs=1) as work,
        ):
            # preload the ACT table set while DMAs run
            warm = const.tile([1, 8], F32)
            nc.vector.memset(warm, 1.0)
            nc.scalar.activation(out=warm, in_=warm,
                                 func=mybir.ActivationFunctionType.Sqrt)
            nc.scalar.activation(out=warm, in_=warm,
                                 func=mybir.ActivationFunctionType.Relu)

            # ---- input DMAs. sync's queue carries the two big streams
            # (xcombo first - it gates prep - then the one-hot rows);
            # scalar's queue carries the small tables in parallel. ----
            ktile = const.tile([128, 4 * W], FP8)
            nc.sync.dma_start(out=ktile, in_=xcombo[:, :])
            taug = const.tile([AUGR, W], BF16)
            nc.scalar.dma_start(out=taug, in_=aug[:, :])
            t_oh = const.tile([128, NT * 128], BF16)
            if T < RB:  # zero the pad rows the gathers skip
                nc.vector.memset(t_oh, 0.0)
            # spread the dma_start issue cost (~0.6-1.8us EACH on the
            # initiating engine) across all three DMA-capable engines:
            # sync: xcombo + early one-hot; gpsimd: late one-hot;
            # scalar: aug + sel halves (parallel queue).
            t_sel = const.tile([128, NT * TP], BF16)
            ohq = (NT + 3) // 4 * 128  # one-hot cols per quarter-chunk
            slq = (NT + 1) // 2 * TP  # sel cols per half-chunk
            for s in range(2):  # gpsimd is otherwise idle; keep scalar
                s0, s1 = s * slq, min((s + 1) * slq, NT * TP)  # free for
                if s0 < s1:  # the prep ACT chain
                    nc.gpsimd.dma_start(out=t_sel[:, s0:s1], in_=selm[:, s0:s1])
            for c in range(4):
                c0, c1 = c * ohq, min((c + 1) * ohq, NT * 128)
                if c0 >= c1:
                    continue
                # chunk 0 gates the first loop tile: route it through the
                # near-empty scalar queue, not behind xcombo on sync's
                eng = (nc.scalar, nc.sync, nc.gpsimd, nc.gpsimd)[c]
                eng.dma_start(out=t_oh[0:T, c0:c1], in_=onehot[0:T, c0:c1])
                eng.dma_start(
                    out=t_oh[RB : RB + T, c0:c1], in_=onehot[T : 2 * T, c0:c1]
                )

            kc = ktile.rearrange("p (c w) -> p c w", c=4)

            rhs_c = work.tile([128, WR], BF16)  # [d_in | bias] combined

            with tc.tile_pool(name="psum_prep", bufs=1, space="PSUM") as psp:
                # ---- g = dot - 0.5*sq_col - 0.5*sq_anchor - class masks ----
                g = psp.tile([128, 1024], F32)
                for n0, n1 in [(0, 512), (512, B)]:
                    for ki in range(4):
                        nc.tensor.matmul(
                            g[:, n0:n1],
                            kc[:, ki, L0:L1],
                            kc[:, ki, n0:n1],
                            start=(ki == 0),
                            stop=False,
                        )
                    nc.tensor.matmul(
                        g[:, n0:n1], taug[:, L0:L1], taug[:, n0:n1],
                        start=False, stop=True,
                    )
                # ---- positive-pair distances: pb[r, t] ----
                pb = psp.tile([128, T], F32)
                for blk in range(2):
                    r0, r1 = blk * RB, (blk + 1) * RB
                    c0, c1 = P0 + blk * T, P0 + (blk + 1) * T
                    for ki in range(4):
                        nc.tensor.matmul(
                            pb[r0:r1, :],
                            kc[:, ki, L0 + r0 : L0 + r1],
                            kc[:, ki, c0:c1],
                            start=(ki == 0),
                            stop=False,
                        )
                    nc.tensor.matmul(
                        pb[r0:r1, :], taug[:, L0 + r0 : L0 + r1],
                        taug[:, c0:c1], start=False, stop=True,
                    )
                # bias = d_pos + margin - SHIFT -> rhs_c[:, B:B+T] (bf16)
                bsq = work.tile([128, T], F32)
                nc.scalar.activation(
                    out=bsq, in_=pb, func=mybir.ActivationFunctionType.Relu,
                    scale=-2.0,
                )
                bd = work.tile([128, T], F32)
                nc.scalar.activation(
                    out=bd, in_=bsq, func=mybir.ActivationFunctionType.Sqrt,
                )
                nc.vector.tensor_scalar(
                    out=rhs_c[:, B : B + T], in0=bd,
                    scalar1=float(MARGIN - SHIFT), scalar2=None,
                    op0=mybir.AluOpType.add,
                )
                # d_in = d - SHIFT (masked columns stay ~306)
                dsq = work.tile([128, B], F32)
                nc.scalar.activation(
                    out=dsq, in_=g[:, 0:B],
                    func=mybir.ActivationFunctionType.Relu, scale=-2.0,
                )
                dmat = work.tile([128, B], F32)
                nc.scalar.activation(
                    out=dmat, in_=dsq, func=mybir.ActivationFunctionType.Sqrt,
                )
                nc.vector.tensor_scalar(
                    out=rhs_c[:, 0:B], in0=dmat,
                    scalar1=-SHIFT, scalar2=None,
                    op0=mybir.AluOpType.add,
                )

            # ---- pair loop (software-pipelined: the count of tile t is
            # emitted during tile t+1 so the in-order DVE stream never
            # waits on ACT) ----
            sum_ps = work.tile([128, NT], F32)
            cnt_ps = work.tile([128, NT], F32)
            scr = work.tile([128, B], BF16)  # throwaway out of the count
            with (
                tc.tile_pool(name="psum_loop", bufs=3, space="PSUM") as psl,
                tc.tile_pool(name="psum_tail", bufs=1, space="PSUM") as pst,
                tc.tile_pool(name="bp", bufs=3) as bpp,
                tc.tile_pool(name="ttrp", bufs=2) as ttrp,
                tc.tile_pool(name="o1p", bufs=3) as o1p,
            ):
                o1s = {}

                def emit_cnt(t):
                    nc.vector.tensor_scalar(
                        out=scr, in0=o1s.pop(t), scalar1=0.0, scalar2=0.0,
                        op0=mybir.AluOpType.is_gt, op1=mybir.AluOpType.add,
                        accum_out=cnt_ps[:, t : t + 1],
                    )

                for tau in range(NT):
                    # PE: replicate anchor row + bias row per pair
                    rep = psl.tile([128, 1024], F32, tag="rep")
                    oh = t_oh[:, tau * 128 : (tau + 1) * 128]
                    nc.tensor.matmul(
                        rep[:, 0:512], oh, rhs_c[:, 0:512],
                        start=True, stop=True,
                    )
                    nc.tensor.matmul(
                        rep[:, 512:WR], oh, rhs_c[:, 512:WR],
                        start=True, stop=True,
                    )
                    # DVE: extract this pair's bias
                    bias_p = bpp.tile([128, 1], F32, tag="bias_p")
                    ttr = ttrp.tile([128, T], F32, tag="ttr")
                    nc.vector.scalar_tensor_tensor(
                        out=ttr, in0=rep[:, B:WR], scalar=1.0,
                        in1=t_sel[:, tau * TP : tau * TP + T],
                        op0=mybir.AluOpType.mult, op1=mybir.AluOpType.mult,
                        accum_out=bias_p,
                    )
                    # ACT: o1 = relu(bias_p - rep) + sum accumulator
                    # (f32 out: the bf16-out + accum combo is ~230ns slower)
                    o1 = o1p.tile([128, B], F32, tag="o1")
                    nc.scalar.activation(
                        out=o1, in_=rep[:, 0:B],
                        func=mybir.ActivationFunctionType.Relu,
                        bias=bias_p, scale=-1.0,
                        accum_out=sum_ps[:, tau : tau + 1],
                    )
                    o1s[tau] = o1
                    if tau >= 1:
                        emit_cnt(tau - 1)
                emit_cnt(NT - 1)

                # ---- tail: reduce -> [128,2] -> ones-matmul -> [1,2] ----
                stat = work.tile([128, 2], F32)
                nc.vector.tensor_reduce(
                    out=stat[:, 0:1], in_=sum_ps, axis=mybir.AxisListType.X,
                    op=mybir.AluOpType.add,
                )
                nc.vector.tensor_reduce(
                    out=stat[:, 1:2], in_=cnt_ps, axis=mybir.AxisListType.X,
                    op=mybir.AluOpType.add,
                )
                onesf = work.tile([128, 1], F32)
                nc.vector.memset(onesf, 1.0)
                tot = pst.tile([1, 2], F32)
                nc.tensor.matmul(tot, onesf, stat, start=True, stop=True)
                tot_s = work.tile([1, 2], F32)
                nc.vector.tensor_copy(tot_s, tot)
                nc.sync.dma_start(out=out_d[:, :], in_=tot_s)

    _split_multi_waits(nc)
    return nc


def _schedule(labels: np.ndarray):
    """Group anchors by class, pair classes onto cores (big with small)."""
    vals, counts = np.unique(labels, return_counts=True)
    classes = [np.nonzero(labels == v)[0] for v in vals]
    order = np.argsort(-counts, kind="stable")
    classes = [classes[i] for i in order]
    sizes = [len(c) for c in classes]
    if len(classes) > 2 * N_CORES or max(sizes) > RB:
        return None  # device path infeasible for this label layout
    while len(classes) < 2 * N_CORES:
        classes.append(np.zeros((0,), dtype=np.int64))
    blocks = []
    for i in range(N_CORES):
        blocks.append((classes[i], classes[2 * N_CORES - 1 - i]))
    T = max(1, max(len(c) for c, _ in blocks))
    npairs = [len(a) * (len(a) - 1) + len(b) * (len(b) - 1) for a, b in blocks]
    NT = max(1, (max(npairs) + 127) // 128)
    return blocks, T, NT


def _host_fallback(X: np.ndarray, labels: np.ndarray) -> np.float32:
    """Exact numpy implementation (only for label layouts the device
    schedule cannot represent — cannot occur for randint(0,16) labels)."""
    Xd = X.astype(np.float64)
    dot = Xd @ Xd.T
    sq = np.diag(dot).copy()
    dm = np.maximum(sq[None, :] - 2.0 * dot + sq[:, None], 0.0)
    zero = dm == 0.0
    dm = np.sqrt(dm + zero * EPS) * (1.0 - zero)
    total = 0.0
    npos = 0
    B = len(labels)
    for i in range(B):
        pos = (labels == labels[i]) & (np.arange(B) != i)
        neg = labels != labels[i]
        p = dm[i, pos] + MARGIN
        n = dm[i, neg]
        tl = np.maximum(p[:, None] - n[None, :], 0.0)
        total += tl.sum()
        npos += (tl > EPS).sum()
    return np.float32(total / (npos + EPS))


def _make_in_maps(X: np.ndarray, lab: np.ndarray, blocks, T: int, NT: int):
    B, D = X.shape
    sq = (X.astype(np.float64) ** 2).sum(axis=1).astype(np.float32)
    W = B + 128 + 2 * T

    X8 = X.astype(NP_FP8)
    XT8 = np.ascontiguousarray(X8.T)
    sq_hi = (-0.5 * sq).astype(NP_BF16)
    sq_lo = ((-0.5 * sq) - sq_hi.astype(np.float32)).astype(NP_BF16)
    in_maps = []
    for core in range(N_CORES):
        cls_a, cls_b = blocks[core]
        row_idx = np.zeros(128, dtype=np.int64)
        for blk, cls in enumerate((cls_a, cls_b)):
            m = len(cls)
            r0 = blk * RB
            if m:
                row_idx[r0 : r0 + m] = cls
                row_idx[r0 + m : r0 + RB] = cls[0]

        par_idx = np.zeros(2 * T, dtype=np.int64)
        for blk, cls in enumerate((cls_a, cls_b)):
            m = len(cls)
            if m:
                par_idx[blk * T : blk * T + m] = cls

        # X part of the contraction (fp8), packed [128, 4, W]
        xcols = np.concatenate([XT8, XT8[:, row_idx], XT8[:, par_idx]], axis=1)
        packed = np.ascontiguousarray(
            xcols.reshape(4, 128, W).transpose(1, 0, 2)
        ).reshape(128, 4 * W)

        # aug rows (bf16): anchor norms (stationary data x moving ones),
        # column norms (stationary ones x moving data), 16 class-mask
        # rows (stationary 240*onehot x moving -240 on own-class columns)
        L0, L1 = B, B + 128
        augt = np.zeros((AUGR, W), dtype=np.float32)
        augt[0:4, :] = 1.0
        augt[0, L0:L1] = sq_hi[row_idx].astype(np.float32)
        augt[1, L0:L1] = (
            -0.5 * sq[row_idx] - sq_hi[row_idx].astype(np.float32)
        )
        augt[2, 0:B] = sq_hi.astype(np.float32)
        augt[3, 0:B] = sq_lo.astype(np.float32)
        augt[2, L1:W] = sq_hi[par_idx].astype(np.float32)
        augt[3, L1:W] = sq_lo[par_idx].astype(np.float32)
        for c in range(N_CLASSES):
            augt[4 + c, 0:B] = np.where(lab == c, -MASKQ, 0.0)
            augt[4 + c, L0:L1] = (lab[row_idx] == c) * MASKQ
        aug_bf = augt.astype(NP_BF16)
        # keep the anchor-norm hi+lo split exact after the bf16 round
        aug_bf[1, L0:L1] = (
            -0.5 * sq[row_idx] - aug_bf[0, L0:L1].astype(np.float32)
        ).astype(NP_BF16)

        # pair tables: one-hot anchor pick (compact: 2*T used rows) and
        # the bias-column select mask
        onehot = np.zeros((2 * T, NT * 128), dtype=NP_BF16)
        selm = np.zeros((128, NT * TP), dtype=NP_BF16)
        p = 0
        for blk, cls in enumerate((cls_a, cls_b)):
            m = len(cls)
            for i in range(m):
                for t in range(m):
                    if t == i:
                        continue
                    tau, q = divmod(p, 128)
                    onehot[blk * T + i, tau * 128 + q] = 1.0
                    selm[q, tau * TP + t] = 1.0
                    p += 1
        assert p <= NT * 128

        in_maps.append(
            {"xcombo": packed, "aug": aug_bf, "selm": selm, "onehot": onehot}
        )
    return in_maps


def kernel(embeddings: np.ndarray, labels: np.ndarray) -> np.ndarray:
    X = np.ascontiguousarray(np.asarray(embeddings), dtype=np.float32)
    lab = np.asarray(labels).astype(np.int64)
    B, D = X.shape
    assert B == B_TOTAL and D == D_MODEL, (B, D)

    sched = _schedule(lab)
    if sched is None:
        return _host_fallback(X, lab)
    blocks, T, NT = sched
    in_maps = _make_in_maps(X, lab, blocks, T, NT)

    key = (B, D, T, NT)
    nc = _PROGRAM_CACHE.get(key)
    if nc is None:
        nc = _build_program(B, D, T, NT)
        _PROGRAM_CACHE[key] = nc

    res = run_bass_kernel_spmd(nc, in_maps, core_ids=list(range(N_CORES)))
    total_sum = 0.0
    total_cnt = 0.0
    for r in res.results:
        o = np.asarray(r["out"], dtype=np.float64)
        total_sum += o[0, 0]
        total_cnt += o[0, 1]
    return np.float32(total_sum / (total_cnt + EPS))


# revision 34
# speedup vs baseline: 1.1443x; 1.1443x over previous
"""Batch-all triplet loss on 8 Trainium2 NeuronCores (Bass/Tile).

Math: with d = pairwise euclidean distance matrix of the B embeddings,
  loss = sum_{i,j,k valid} relu(d[i,j] - d[i,k] + margin) / (#positive + eps)
valid <=> i != j, labels[i] == labels[j], labels[i] != labels[k]
(the other distinctness constraints are implied by the label ones).

Sharding: anchors are grouped by class; each core hosts 2 classes in two
64-row blocks (data-driven gathers keep the single SPMD program uniform).

Per core, on device:
  prep:
  - one fused matmul group produces g[r, c] = dot(x_r, x_c) - 0.5*sq_c
    - 0.5*sq_r - MASK*[class(c)==class(r)] : the X contraction runs in
    fp8 (4x128 K-chunks; the stochastic fp8 dot error ~0.02 on d is far
    inside the 2e-2 gate and halves the gating DMA bytes); a 20-row bf16
    tail chunk carries the anchor squared norm (hi+lo, stationary data
    vs moving ones), the column squared norm (hi+lo, moving data vs
    stationary ones) and 16 class-mask rows (+-240 products).
  - ACT Relu(scale=-2) then ACT Sqrt gives dmat (masked columns ~338);
    DVE tensor_scalar(-SHIFT) emits rhs_c[:, 0:B] (bf16, recentred).
  - the same structure over the partner columns gives bias[r, t] =
    d(anchor_r, t-th member of r's class) + margin - SHIFT at
    rhs_c[:, B:B+T] (bf16).
  pair loop, NT tiles of 128 (anchor,positive) pairs (software-pipelined
  so each engine's in-order stream never stalls):
  - PE replicates each pair's anchor row + bias row with a one-hot
    matmul into PSUM: rep[p, 0:B+T] = rhs_c[anchor(p), :].
  - DVE builds the bias-select mask on the fly (sel[p,t] =
    (t == pos_idx[p, tau]), 4x mode) -> scalar_tensor_tensor extracts
    bias_p; ACT Relu(scale=-1, bias=bias_p) -> o1 (bf16 SBUF);
    DVE is_gt -> o2 (bf16, 4x mode).
  - PE reduces o1 and o2 with ones-matmuls into two persistent PSUM
    accumulator regions (column-wrapped, one accumulation group each
    across all NT tiles) - the DVE reduce path is 1x-capped and slower.
  - invalid negatives contribute 0 (the +306 mask), padded pairs are
    all-zero rows with bias 0 and contribute 0 to both sums.
  tail: one strided tensor_reduce of the [1, 2x512] accumulator ->
  [1, 2] (sum, count) DMA'd out; host adds the 8 cores up.
"""

import numpy as np

import bass_rust
import concourse.bass as bass
import concourse.tile as tile
from concourse import mybir
from concourse.bass_utils import run_bass_kernel_spmd

N_CORES = 8
D_MODEL = 512
B_TOTAL = 640
N_CLASSES = 16
MARGIN = 0.3
EPS = 1e-8
RB = 64  # rows per block (max class size the device path supports)
MASKQ = 240.0  # class-mask factor; product 57600 pushes masked d to ~338
SHIFT = 32.0  # d values live in ~[27.6, 37]; recentring helps bf16
AUGR = 4 + N_CLASSES  # sqa_hi, sqa_lo, norm_hi, norm_lo, class masks
TP = 48  # padded iota/sel width (even for DVE 4x mode)
F32 = mybir.dt.float32
BF16 = mybir.dt.bfloat16
FP8 = mybir.dt.float8e4
NP_BF16 = mybir.dt.np(mybir.dt.bfloat16)
NP_FP8 = mybir.dt.np(mybir.dt.float8e4)

_PROGRAM_CACHE: dict = {}


def _split_multi_waits(nc):
    """This toolchain's walrus codegen supports only ONE sync-wait per
    instruction; Tile can emit several. Move the extra waits onto
    same-engine NoOps inserted immediately before the instruction."""
    for func in nc.m.functions:
        for block in func.blocks:
            out = []
            for inst in block.instructions:
                si = inst.sync_info
                waits = list(si.on_wait) if si else []
                if len(waits) > 1:
                    for j, w in enumerate(waits[:-1]):
                        nop = mybir.InstNoOp(
                            name=f"{inst.name}-wsplit{j}", ins=[], outs=[]
                        )
                        nop.engine = inst.engine
                        nop.sync_info = bass_rust.SyncInfo(on_wait=[w], on_update=[])
                        out.append(nop)
                    inst.sync_info = bass_rust.SyncInfo(
                        on_wait=[waits[-1]], on_update=list(si.on_update)
                    )
                out.append(inst)
            block.instructions = out


def _build_program(B: int, D: int, T: int, NT: int):
    """One SPMD program for all 8 cores; per-core behavior is data-driven."""
    nc = bass.Bass()

    # xcombo columns: [0:B) moving operand (X^T), [B:B+128) stationary
    # anchor gather, [B+128:B+128+2T) positive-partner gather, all fp8.
    # Packed DRAM layout [128, 4*W]: chunk c of the K=512 contraction
    # occupies cols [c*W:(c+1)*W) (K-row c*128+p lives in partition p).
    W = B + 128 + 2 * T
    xcombo = nc.declare_dram_parameter("xcombo", [128, 4 * W], FP8, isOutput=False)
    aug = nc.declare_dram_parameter("aug", [AUGR, W], BF16, isOutput=False)
    # one-hot stationary, shipped with only the 2*T meaningful rows
    onehot = nc.declare_dram_parameter(
        "onehot", [2 * T, NT * 128], BF16, isOutput=False
    )
    selm = nc.declare_dram_parameter("selm", [128, NT * TP], BF16, isOutput=False)
    out_d = nc.declare_dram_parameter("out", [1, 2], F32, isOutput=True)

    WR = B + T  # replicated tile: negative columns + bias columns
    L0, L1 = B, B + 128  # stationary (anchor) columns within a chunk
    P0 = B + 128  # partner columns within a chunk

    with tile.TileContext(nc) as tc:
        with (
            tc.tile_pool(name="const", bufs=1) as const,
            tc.tile_pool(name="work", bufs=1) as work,
        ):
            # preload the ACT table set while DMAs run
            warm = const.tile([1, 8], F32)
            nc.vector.memset(warm, 1.0)
            nc.scalar.activation(out=warm, in_=warm,
                                 func=mybir.ActivationFunctionType.Sqrt)
            nc.scalar.activation(out=warm, in_=warm,
                                 func=mybir.ActivationFunctionType.Relu)

            # ---- input DMAs. sync's queue carries the two big streams
            # (xcombo first - it gates prep - then the one-hot rows);
            # scalar's queue carries the small tables in parallel. ----
            ktile = const.tile([128, 4 * W], FP8)
            nc.sync.dma_start(out=ktile, in_=xcombo[:, :])
            taug = const.tile([AUGR, W], BF16)
            nc.scalar.dma_start(out=taug, in_=aug[:, :])
            # One tile PER chunk: a multi-writer tile makes every reader
            # wait for ALL of its writers, so a shared t_oh would gate
            # the first pair tile on the LAST one-hot DMA. Issue cost is
            # spread over the three DMA-capable engines (sync: xcombo +
            # early one-hot; gpsimd: sel + late one-hot; scalar: aug).
            ohq = (NT + 3) // 4 * 128  # one-hot cols per quarter-chunk
            slq = (NT + 1) // 2 * TP  # sel cols per half-chunk
            t_sels, t_ohs = [], []
            for s in range(2):
                s0, s1 = s * slq, min((s + 1) * slq, NT * TP)
                ts_ = const.tile([128, max(1, s1 - s0)], BF16, tag=f"sel{s}")
                t_sels.append(ts_)
                if s0 < s1:
                    nc.gpsimd.dma_start(out=ts_, in_=selm[:, s0:s1])
            for c in range(4):
                c0, c1 = c * ohq, min((c + 1) * ohq, NT * 128)
                to_ = const.tile([128, max(1, c1 - c0)], BF16, tag=f"oh{c}")
                t_ohs.append(to_)
                if c0 >= c1:
                    continue
                if T < RB:  # zero the pad rows the gathers skip
                    nc.vector.memset(to_, 0.0)
                eng = nc.sync if c < 2 else nc.gpsimd
                eng.dma_start(out=to_[0:T, :], in_=onehot[0:T, c0:c1])
                eng.dma_start(
                    out=to_[RB : RB + T, :], in_=onehot[T : 2 * T, c0:c1]
                )

            kc = ktile.rearrange("p (c w) -> p c w", c=4)

            rhs_c = work.tile([128, WR], BF16)  # [d_in | bias] combined

            with tc.tile_pool(name="psum_prep", bufs=1, space="PSUM") as psp:
                # ---- g = dot - 0.5*sq_col - 0.5*sq_anchor - class masks ----
                g = psp.tile([128, 1024], F32)
                for n0, n1 in [(0, 512), (512, B)]:
                    for ki in range(4):
                        nc.tensor.matmul(
                            g[:, n0:n1],
                            kc[:, ki, L0:L1],
                            kc[:, ki, n0:n1],
                            start=(ki == 0),
                            stop=False,
                        )
                    nc.tensor.matmul(
                        g[:, n0:n1], taug[:, L0:L1], taug[:, n0:n1],
                        start=False, stop=True,
                    )
                # ---- positive-pair distances: pb[r, t] ----
                pb = psp.tile([128, T], F32)
                for blk in range(2):
                    r0, r1 = blk * RB, (blk + 1) * RB
                    c0, c1 = P0 + blk * T, P0 + (blk + 1) * T
                    for ki in range(4):
                        nc.tensor.matmul(
                            pb[r0:r1, :],
                            kc[:, ki, L0 + r0 : L0 + r1],
                            kc[:, ki, c0:c1],
                            start=(ki == 0),
                            stop=False,
                        )
                    nc.tensor.matmul(
                        pb[r0:r1, :], taug[:, L0 + r0 : L0 + r1],
                        taug[:, c0:c1], start=False, stop=True,
                    )
                # bias = d_pos + margin - SHIFT -> rhs_c[:, B:B+T] (bf16)
                bsq = work.tile([128, T], F32)
                nc.scalar.activation(
                    out=bsq, in_=pb, func=mybir.ActivationFunctionType.Relu,
                    scale=-2.0,
                )
                bd = work.tile([128, T], F32)
                nc.scalar.activation(
                    out=bd, in_=bsq, func=mybir.ActivationFunctionType.Sqrt,
                )
                nc.vector.tensor_scalar(
                    out=rhs_c[:, B : B + T], in0=bd,
                    scalar1=float(MARGIN - SHIFT), scalar2=None,
                    op0=mybir.AluOpType.add,
                )
                # d_in = d - SHIFT (masked columns stay ~306)
                dsq = work.tile([128, B], F32)
                nc.scalar.activation(
                    out=dsq, in_=g[:, 0:B],
                    func=mybir.ActivationFunctionType.Relu, scale=-2.0,
                )
                dmat = work.tile([128, B], F32)
                nc.scalar.activation(
                    out=dmat, in_=dsq, func=mybir.ActivationFunctionType.Sqrt,
                )
                nc.vector.tensor_scalar(
                    out=rhs_c[:, 0:B], in0=dmat,
                    scalar1=-SHIFT, scalar2=None,
                    op0=mybir.AluOpType.add,
                )

            # ---- pair loop (software-pipelined: the count of tile t is
            # emitted during tile t+1 so the in-order DVE stream never
            # waits on ACT) ----
            sum_ps = work.tile([128, NT], F32)
            cnt_ps = work.tile([128, NT], F32)
            scr = work.tile([128, B], BF16)  # throwaway out of the count
            with (
                tc.tile_pool(name="psum_loop", bufs=3, space="PSUM") as psl,
                tc.tile_pool(name="psum_tail", bufs=1, space="PSUM") as pst,
                tc.tile_pool(name="bp", bufs=3) as bpp,
                tc.tile_pool(name="ttrp", bufs=2) as ttrp,
                tc.tile_pool(name="o1p", bufs=3) as o1p,
            ):
                o1s = {}

                def emit_cnt(t):
                    nc.vector.tensor_scalar(
                        out=scr, in0=o1s.pop(t), scalar1=0.0, scalar2=0.0,
                        op0=mybir.AluOpType.is_gt, op1=mybir.AluOpType.add,
                        accum_out=cnt_ps[:, t : t + 1],
                    )

                tpq = ohq // 128  # taus per one-hot chunk
                tps = slq // TP  # taus per sel chunk
                for tau in range(NT):
                    # PE: replicate anchor row + bias row per pair
                    rep = psl.tile([128, 1024], F32, tag="rep")
                    oh = t_ohs[tau // tpq][
                        :, (tau % tpq) * 128 : (tau % tpq + 1) * 128
                    ]
                    nc.tensor.matmul(
                        rep[:, 0:512], oh, rhs_c[:, 0:512],
                        start=True, stop=True,
                    )
                    nc.tensor.matmul(
                        rep[:, 512:WR], oh, rhs_c[:, 512:WR],
                        start=True, stop=True,
                    )
                    # DVE: extract this pair's bias
                    bias_p = bpp.tile([128, 1], F32, tag="bias_p")
                    ttr = ttrp.tile([128, T], F32, tag="ttr")
                    nc.vector.scalar_tensor_tensor(
                        out=ttr, in0=rep[:, B:WR], scalar=1.0,
                        in1=t_sels[tau // tps][
                            :, (tau % tps) * TP : (tau % tps) * TP + T
                        ],
                        op0=mybir.AluOpType.mult, op1=mybir.AluOpType.mult,
                        accum_out=bias_p,
                    )
                    # ACT: o1 = relu(bias_p - rep) + sum accumulator
                    # (f32 out: the bf16-out + accum combo is ~230ns slower)
                    o1 = o1p.tile([128, B], F32, tag="o1")
                    nc.scalar.activation(
                        out=o1, in_=rep[:, 0:B],
                        func=mybir.ActivationFunctionType.Relu,
                        bias=bias_p, scale=-1.0,
                        accum_out=sum_ps[:, tau : tau + 1],
                    )
                    o1s[tau] = o1
                    if tau >= 1:
                        emit_cnt(tau - 1)
                emit_cnt(NT - 1)

                # ---- tail: reduce -> [128,2] -> ones-matmul -> [1,2] ----
                stat = work.tile([128, 2], F32)
                nc.vector.tensor_reduce(
                    out=stat[:, 0:1], in_=sum_ps, axis=mybir.AxisListType.X,
                    op=mybir.AluOpType.add,
                )
                nc.vector.tensor_reduce(
                    out=stat[:, 1:2], in_=cnt_ps, axis=mybir.AxisListType.X,
                    op=mybir.AluOpType.add,
                )
                onesf = work.tile([128, 1], F32)
                nc.vector.memset(onesf, 1.0)
                tot = pst.tile([1, 2], F32)
                nc.tensor.matmul(tot, onesf, stat, start=True, stop=True)
                tot_s = work.tile([1, 2], F32)
                nc.vector.tensor_copy(tot_s, tot)
                nc.sync.dma_start(out=out_d[:, :], in_=tot_s)

    _split_multi_waits(nc)
    return nc


def _schedule(labels: np.ndarray):
    """Group anchors by class, pair classes onto cores (big with small)."""
    vals, counts = np.unique(labels, return_counts=True)
    classes = [np.nonzero(labels == v)[0] for v in vals]
    order = np.argsort(-counts, kind="stable")
    classes = [classes[i] for i in order]
    sizes = [len(c) for c in classes]
    if len(classes) > 2 * N_CORES or max(sizes) > RB:
        return None  # device path infeasible for this label layout
    while len(classes) < 2 * N_CORES:
        classes.append(np.zeros((0,), dtype=np.int64))
    blocks = []
    for i in range(N_CORES):
        blocks.append((classes[i], classes[2 * N_CORES - 1 - i]))
    T = max(1, max(len(c) for c, _ in blocks))
    npairs = [len(a) * (len(a) - 1) + len(b) * (len(b) - 1) for a, b in blocks]
    NT = max(1, (max(npairs) + 127) // 128)
    return blocks, T, NT


def _host_fallback(X: np.ndarray, labels: np.ndarray) -> np.float32:
    """Exact numpy implementation (only for label layouts the device
    schedule cannot represent — cannot occur for randint(0,16) labels)."""
    Xd = X.astype(np.float64)
    dot = Xd @ Xd.T
    sq = np.diag(dot).copy()
    dm = np.maximum(sq[None, :] - 2.0 * dot + sq[:, None], 0.0)
    zero = dm == 0.0
    dm = np.sqrt(dm + zero * EPS) * (1.0 - zero)
    total = 0.0
    npos = 0
    B = len(labels)
    for i in range(B):
        pos = (labels == labels[i]) & (np.arange(B) != i)
        neg = labels != labels[i]
        p = dm[i, pos] + MARGIN
        n = dm[i, neg]
        tl = np.maximum(p[:, None] - n[None, :], 0.0)
        total += tl.sum()
        npos += (tl > EPS).sum()
    return np.float32(total / (npos + EPS))


def _make_in_maps(X: np.ndarray, lab: np.ndarray, blocks, T: int, NT: int):
    B, D = X.shape
    sq = (X.astype(np.float64) ** 2).sum(axis=1).astype(np.float32)
    W = B + 128 + 2 * T

    X8 = X.astype(NP_FP8)
    XT8 = np.ascontiguousarray(X8.T)
    sq_hi = (-0.5 * sq).astype(NP_BF16)
    sq_lo = ((-0.5 * sq) - sq_hi.astype(np.float32)).astype(NP_BF16)
    in_maps = []
    for core in range(N_CORES):
        cls_a, cls_b = blocks[core]
        row_idx = np.zeros(128, dtype=np.int64)
        for blk, cls in enumerate((cls_a, cls_b)):
            m = len(cls)
            r0 = blk * RB
            if m:
                row_idx[r0 : r0 + m] = cls
                row_idx[r0 + m : r0 + RB] = cls[0]

        par_idx = np.zeros(2 * T, dtype=np.int64)
        for blk, cls in enumerate((cls_a, cls_b)):
            m = len(cls)
            if m:
                par_idx[blk * T : blk * T + m] = cls

        # X part of the contraction (fp8), packed [128, 4, W]
        xcols = np.concatenate([XT8, XT8[:, row_idx], XT8[:, par_idx]], axis=1)
        packed = np.ascontiguousarray(
            xcols.reshape(4, 128, W).transpose(1, 0, 2)
        ).reshape(128, 4 * W)

        # aug rows (bf16): anchor norms (stationary data x moving ones),
        # column norms (stationary ones x moving data), 16 class-mask
        # rows (stationary 240*onehot x moving -240 on own-class columns)
        L0, L1 = B, B + 128
        augt = np.zeros((AUGR, W), dtype=np.float32)
        augt[0:4, :] = 1.0
        augt[0, L0:L1] = sq_hi[row_idx].astype(np.float32)
        augt[1, L0:L1] = (
            -0.5 * sq[row_idx] - sq_hi[row_idx].astype(np.float32)
        )
        augt[2, 0:B] = sq_hi.astype(np.float32)
        augt[3, 0:B] = sq_lo.astype(np.float32)
        augt[2, L1:W] = sq_hi[par_idx].astype(np.float32)
        augt[3, L1:W] = sq_lo[par_idx].astype(np.float32)
        for c in range(N_CLASSES):
            augt[4 + c, 0:B] = np.where(lab == c, -MASKQ, 0.0)
            augt[4 + c, L0:L1] = (lab[row_idx] == c) * MASKQ
        aug_bf = augt.astype(NP_BF16)
        # keep the anchor-norm hi+lo split exact after the bf16 round
        aug_bf[1, L0:L1] = (
            -0.5 * sq[row_idx] - aug_bf[0, L0:L1].astype(np.float32)
        ).astype(NP_BF16)

        # pair tables: one-hot anchor pick (compact: 2*T used rows) and
        # the bias-column select mask
        onehot = np.zeros((2 * T, NT * 128), dtype=NP_BF16)
        selm = np.zeros((128, NT * TP), dtype=NP_BF16)
        p = 0
        for blk, cls in enumerate((cls_a, cls_b)):
            m = len(cls)
            for i in range(m):
                for t in range(m):
                    if t == i:
                        continue
                    tau, q = divmod(p, 128)
                    onehot[blk * T + i, tau * 128 + q] = 1.0
                    selm[q, tau * TP + t] = 1.0
                    p += 1
        assert p <= NT * 128

        in_maps.append(
            {"xcombo": packed, "aug": aug_bf, "selm": selm, "onehot": onehot}
        )
    return in_maps


def kernel(embeddings: np.ndarray, labels: np.ndarray) -> np.ndarray:
    X = np.ascontiguousarray(np.asarray(embeddings), dtype=np.float32)
    lab = np.asarray(labels).astype(np.int64)
    B, D = X.shape
    assert B == B_TOTAL and D == D_MODEL, (B, D)

    sched = _schedule(lab)
    if sched is None:
        return _host_fallback(X, lab)
    blocks, T, NT = sched
    in_maps = _make_in_maps(X, lab, blocks, T, NT)

    key = (B, D, T, NT)
    nc = _PROGRAM_CACHE.get(key)
    if nc is None:
        nc = _build_program(B, D, T, NT)
        _PROGRAM_CACHE[key] = nc

    res = run_bass_kernel_spmd(nc, in_maps, core_ids=list(range(N_CORES)))
    total_sum = 0.0
    total_cnt = 0.0
    for r in res.results:
        o = np.asarray(r["out"], dtype=np.float64)
        total_sum += o[0, 0]
        total_cnt += o[0, 1]
    return np.float32(total_sum / (total_cnt + EPS))
